# revision 2
# baseline (speedup 1.0000x reference)
"""Sliding-window GQA attention (soft-capped) on 8 TRN2 NeuronCores.

Problem: B=2, S=2048, H=32 q-heads, H_KV=8 kv-heads, D=128, causal sliding
window 1024, logits soft-cap 30*tanh(s/30), scale 1/sqrt(D).

Sharding: head-parallel. Core c gets kv head c and q heads [4c, 4c+4) —
fully independent per core, no collectives.

Per-core algorithm (all on one NeuronCore, Tile-scheduled):
  - Q^T/K^T layouts ([d, s]) built on-chip via PE transposes, cast to bf16.
  - Scores computed TRANSPOSED: for each k-tile kt, one strip
    S^T[k=128, q window <=1152] = K_tile^T.T @ Q^T — avoids transposing
    probabilities for the PV matmul.
  - Soft-cap+softmax without max-subtraction (logits bounded by +-30):
    t = tanh(s * scale/30) on ScalarE (PSUM->SBUF), E = exp(30 t) on
    ScalarE (-> bf16). Causal/window masks: multiply 2 boundary 128-col
    blocks by 0/1 masks on VectorE.
  - For each 512-wide q-chunk: num^T[d, q] = sum_kt V_kt.T.T @ E_strip
    accumulated in PSUM (per-element has_written handles the staggered
    strip windows); den[q] replicated across partitions via an all-ones
    stationary matmul. out = (num/den) transposed back via PE.
"""

import numpy as np

import concourse.bass as bass
import concourse.mybir as mybir
import concourse.tile as tile
from concourse import bacc
from concourse.bass_utils import run_bass_kernel_spmd
from concourse.masks import make_identity

AF = mybir.ActivationFunctionType
F32 = mybir.dt.float32
BF16 = mybir.dt.bfloat16

P = 128  # head dim == partition count == seq tile
B = 2
S = 2048
QH = 4  # q heads per core
NT = S // P  # 16 seq tiles
W = 1024  # sliding window
MAXW = W + P  # max strip width (9 tiles)
CHUNK = 512
NCH = S // CHUNK  # q-chunks per (b, head)
SCALE = 1.0 / np.sqrt(128.0)
CAP = 30.0
N_CORES = 8


def _strip_width(kt: int) -> int:
    return min(MAXW, S - kt * P)


def build_core_graph():
    nc = bacc.Bacc("TRN2", target_bir_lowering=False, debug=False, num_devices=N_CORES)
    q_ext = nc.declare_dram_parameter("query", [B, S, QH * P], F32, isOutput=False)
    k_ext = nc.declare_dram_parameter("key", [B, S, P], F32, isOutput=False)
    v_ext = nc.declare_dram_parameter("value", [B, S, P], F32, isOutput=False)
    out_ext = nc.declare_dram_parameter("out", [B, S, QH * P], F32, isOutput=True)

    with tile.TileContext(nc) as tc:
        with (
            tc.tile_pool(name="const", bufs=1) as constp,
            tc.tile_pool(name="persist", bufs=1) as pp,
        ):
            ident = constp.tile([P, P], F32, name="ident", tag="ident")
            make_identity(nc, ident[:])
            ones_bf = constp.tile([P, P], BF16, name="ones", tag="ones")
            nc.vector.memset(ones_bf[:], 1.0)
            # Strip coords: row r = k offset, col c = q offset (q-k = c-r).
            # m1 (first 128 cols): keep c >= r (causal).
            m1 = constp.tile([P, P], BF16, name="m1", tag="m1")
            nc.gpsimd.memset(m1[:], 1.0)
            nc.gpsimd.affine_select(
                out=m1[:],
                in_=m1[:],
                compare_op=mybir.AluOpType.is_ge,
                fill=0.0,
                base=0,
                pattern=[[1, P]],
                channel_multiplier=-1,
            )
            # m2 (cols [1024,1152)): keep c' < r (window cutoff at c-r=1024).
            m2 = constp.tile([P, P], BF16, name="m2", tag="m2")
            nc.gpsimd.memset(m2[:], 1.0)
            nc.gpsimd.affine_select(
                out=m2[:],
                in_=m2[:],
                compare_op=mybir.AluOpType.is_gt,
                fill=0.0,
                base=0,
                pattern=[[-1, P]],
                channel_multiplier=1,
            )

            # Persistent bf16 layouts.
            qT = [
                [pp.tile([P, S], BF16, name=f"qT{b}_{h}", tag=f"qT{b}_{h}") for h in range(QH)]
                for b in range(B)
            ]
            kT = [pp.tile([P, S], BF16, name=f"kT{b}", tag=f"kT{b}") for b in range(B)]
            vB = [pp.tile([P, S], BF16, name=f"vB{b}", tag=f"vB{b}") for b in range(B)]

            # ---- prologue: load + transpose/cast ----
            with (
                tc.tile_pool(name="load", bufs=4) as loadp,
                tc.tile_pool(name="tpsum", bufs=2, space="PSUM") as tpp,
            ):
                for b in range(B):
                    for t in range(NT):
                        rows = slice(t * P, (t + 1) * P)
                        cols = slice(t * P, (t + 1) * P)
                        ktile = loadp.tile([P, P], F32, name="kload", tag="kload")
                        nc.sync.dma_start(out=ktile[:], in_=k_ext[b, rows, :])
                        kps = tpp.tile([P, P], F32, name="tp", tag="tp")
                        nc.tensor.transpose(kps[:], ktile[:], ident[:])
                        nc.vector.tensor_copy(kT[b][:, cols], kps[:])

                        vtile = loadp.tile([P, P], F32, name="vload", tag="vload")
                        nc.sync.dma_start(out=vtile[:], in_=v_ext[b, rows, :])
                        nc.vector.tensor_copy(vB[b][:, cols], vtile[:])

                        qtile = loadp.tile([P, QH * P], F32, name="qload", tag="qload")
                        nc.sync.dma_start(out=qtile[:], in_=q_ext[b, rows, :])
                        for h in range(QH):
                            qps = tpp.tile([P, P], F32, name="tp", tag="tp")
                            nc.tensor.transpose(
                                qps[:], qtile[:, h * P : (h + 1) * P], ident[:]
                            )
                            nc.vector.tensor_copy(qT[b][h][:, cols], qps[:])

            # ---- main loop ----
            with (
                tc.tile_pool(name="spsum", bufs=2, space="PSUM") as sp,
                tc.tile_pool(name="apsum", bufs=2, space="PSUM") as auxp,
                tc.tile_pool(name="tbuf", bufs=2) as tbp,
                tc.tile_pool(name="ebuf", bufs=13) as ebp,
                tc.tile_pool(name="misc", bufs=2) as mp,
            ):
                for b in range(B):
                    for h in range(QH):
                        estrips = {}
                        for kt in range(NT):
                            w = _strip_width(kt)
                            q0s = kt * P  # strip q origin
                            strip = sp.tile([P, MAXW], F32, name="strip", tag="strip")
                            for c0 in range(0, w, CHUNK):
                                c1 = min(c0 + CHUNK, w)
                                nc.tensor.matmul(
                                    strip[:, c0:c1],
                                    lhsT=kT[b][:, q0s : q0s + P],
                                    rhs=qT[b][h][:, q0s + c0 : q0s + c1],
                                    start=True,
                                    stop=True,
                                )
                            tstrip = tbp.tile([P, MAXW], F32, name="t", tag="t")
                            nc.scalar.activation(
                                tstrip[:, :w], strip[:, :w], AF.Tanh, scale=SCALE / CAP
                            )
                            estrip = ebp.tile([P, MAXW], BF16, name="e", tag="e")
                            nc.scalar.activation(
                                estrip[:, :w], tstrip[:, :w], AF.Exp, scale=CAP
                            )
                            nc.vector.tensor_mul(estrip[:, 0:P], estrip[:, 0:P], m1[:])
                            if w > W:
                                nc.vector.tensor_mul(
                                    estrip[:, W : W + P], estrip[:, W : W + P], m2[:]
                                )
                            estrips[kt] = estrip

                            if kt % 4 != 3:
                                continue
                            # q-chunk complete
                            c = kt // 4
                            q0 = c * CHUNK
                            kts = list(range(max(0, 4 * c - 8), kt + 1))
                            num = auxp.tile([P, CHUNK], F32, name="aux", tag="aux")
                            den = auxp.tile([P, CHUNK], F32, name="aux", tag="aux")
                            for which, dst in (("num", num), ("den", den)):
                                for i, k2 in enumerate(kts):
                                    s0 = max(q0, k2 * P)
                                    s1 = min(q0 + CHUNK, k2 * P + _strip_width(k2))
                                    col0 = s0 - k2 * P
                                    n = s1 - s0
                                    d0 = s0 - q0
                                    lhs = (
                                        vB[b][:, k2 * P : (k2 + 1) * P]
                                        if which == "num"
                                        else ones_bf[:]
                                    )
                                    nc.tensor.matmul(
                                        dst[:, d0 : d0 + n],
                                        lhsT=lhs,
                                        rhs=estrips[k2][:, col0 : col0 + n],
                                        start=(i == 0),
                                        stop=(i == len(kts) - 1),
                                    )
                            recip = mp.tile([P, CHUNK], F32, name="recip", tag="recip")
                            nc.vector.reciprocal(recip[:], den[:])
                            divided = mp.tile([P, CHUNK], F32, name="div", tag="div")
                            nc.vector.tensor_mul(divided[:], num[:], recip[:])
                            outps = auxp.tile([P, CHUNK], F32, name="aux", tag="aux")
                            for i in range(CHUNK // P):
                                nc.tensor.transpose(
                                    outps[:, i * P : (i + 1) * P],
                                    divided[:, i * P : (i + 1) * P],
                                    ident[:],
                                )
                            ostage = mp.tile([P, CHUNK], F32, name="ostage", tag="ostage")
                            nc.vector.tensor_copy(ostage[:], outps[:])
                            for i in range(CHUNK // P):
                                nc.sync.dma_start(
                                    out=out_ext[
                                        b,
                                        q0 + i * P : q0 + (i + 1) * P,
                                        h * P : (h + 1) * P,
                                    ],
                                    in_=ostage[:, i * P : (i + 1) * P],
                                )
    nc.compile()
    return nc


_NC_CACHE = [None]


def _get_nc():
    if _NC_CACHE[0] is None:
        _NC_CACHE[0] = build_core_graph()
    return _NC_CACHE[0]


def _shard(query, key, value):
    in_maps = []
    for c in range(N_CORES):
        in_maps.append(
            {
                "query": np.ascontiguousarray(
                    query[:, :, c * QH * P : (c + 1) * QH * P], dtype=np.float32
                ),
                "key": np.ascontiguousarray(
                    key[:, :, c * P : (c + 1) * P], dtype=np.float32
                ),
                "value": np.ascontiguousarray(
                    value[:, :, c * P : (c + 1) * P], dtype=np.float32
                ),
            }
        )
    return in_maps


def _run(query, key, value, trace=False):
    nc = _get_nc()
    in_maps = _shard(query, key, value)
    res = run_bass_kernel_spmd(nc, in_maps, core_ids=list(range(N_CORES)), trace=trace)
    out = np.empty((B, S, N_CORES * QH * P), dtype=np.float32)
    for c in range(N_CORES):
        out[:, :, c * QH * P : (c + 1) * QH * P] = res.results[c]["out"]
    return out, res


def kernel(query, key, value):
    out, _ = _run(query, key, value, trace=False)
    return out


# revision 5
# speedup vs baseline: 1.2327x; 1.2327x over previous
"""Sliding-window GQA attention (soft-capped) on 8 TRN2 NeuronCores.

Problem: B=2, S=2048, H=32 q-heads, H_KV=8 kv-heads, D=128, causal sliding
window 1024, logits soft-cap 30*tanh(s/30), scale 1/sqrt(D).

Sharding: head-parallel. Core c gets kv head c and q heads [4c, 4c+4) —
fully independent per core, no collectives.

Per-core algorithm (all on one NeuronCore, Tile-scheduled):
  - Q^T/K^T layouts ([d, s]) built on-chip via PE transposes, cast to bf16.
  - Scores computed TRANSPOSED: for each k-tile kt, one strip
    S^T[k=128, q window <=1152] = K_tile^T.T @ Q^T — avoids transposing
    probabilities for the PV matmul.
  - Soft-cap+softmax without max-subtraction (logits bounded by +-30):
    t = tanh(s * scale/30) on ScalarE (PSUM->SBUF), E = exp(30 t) on
    ScalarE (-> bf16). Causal/window masks: multiply 2 boundary 128-col
    blocks by 0/1 masks on VectorE.
  - For each 512-wide q-chunk: num^T[d, q] = sum_kt V_kt.T.T @ E_strip
    accumulated in PSUM (per-element has_written handles the staggered
    strip windows); den[q] replicated across partitions via an all-ones
    stationary matmul. out = (num/den) transposed back via PE.
"""

import numpy as np

import concourse.bass as bass
import concourse.mybir as mybir
import concourse.tile as tile
from concourse import bacc
from concourse.bass_utils import run_bass_kernel_spmd
from concourse.masks import make_identity

AF = mybir.ActivationFunctionType
F32 = mybir.dt.float32
BF16 = mybir.dt.bfloat16

P = 128  # head dim == partition count == seq tile
B = 2
S = 2048
QH = 4  # q heads per core
NT = S // P  # 16 seq tiles
W = 1024  # sliding window
MAXW = W + P  # max strip width (9 tiles)
CHUNK = 512
NCH = S // CHUNK  # q-chunks per (b, head)
SCALE = 1.0 / np.sqrt(128.0)
CAP = 30.0
N_CORES = 8


def _strip_width(kt: int) -> int:
    return min(MAXW, S - kt * P)


def build_core_graph():
    nc = bacc.Bacc("TRN2", target_bir_lowering=False, debug=False, num_devices=N_CORES)
    q_ext = nc.declare_dram_parameter("query", [B, S, QH * P], F32, isOutput=False)
    k_ext = nc.declare_dram_parameter("key", [B, S, P], F32, isOutput=False)
    v_ext = nc.declare_dram_parameter("value", [B, S, P], F32, isOutput=False)
    out_ext = nc.declare_dram_parameter("out", [B, S, QH * P], F32, isOutput=True)

    with tile.TileContext(nc) as tc:
        with (
            tc.tile_pool(name="const", bufs=1) as constp,
            tc.tile_pool(name="persist", bufs=1) as pp,
        ):
            ident = constp.tile([P, P], F32, name="ident", tag="ident")
            make_identity(nc, ident[:])
            # 32 replicated rows are enough for the den-transpose trick and
            # make the stationary LDWEIGHTS 4x cheaper than a full [128,128].
            ones_bf = constp.tile([P, 32], BF16, name="ones", tag="ones")
            nc.vector.memset(ones_bf[:], 1.0)
            # Strip coords: row r = k offset, col c = q offset (q-k = c-r).
            # m1 (first 128 cols): keep c >= r (causal).
            m1 = constp.tile([P, P], BF16, name="m1", tag="m1")
            nc.gpsimd.memset(m1[:], 1.0)
            nc.gpsimd.affine_select(
                out=m1[:],
                in_=m1[:],
                compare_op=mybir.AluOpType.is_ge,
                fill=0.0,
                base=0,
                pattern=[[1, P]],
                channel_multiplier=-1,
            )
            # m2 (cols [1024,1152)): keep c' < r (window cutoff at c-r=1024).
            m2 = constp.tile([P, P], BF16, name="m2", tag="m2")
            nc.gpsimd.memset(m2[:], 1.0)
            nc.gpsimd.affine_select(
                out=m2[:],
                in_=m2[:],
                compare_op=mybir.AluOpType.is_gt,
                fill=0.0,
                base=0,
                pattern=[[-1, P]],
                channel_multiplier=1,
            )

            # Persistent bf16 layouts.
            qT = [
                [pp.tile([P, S], BF16, name=f"qT{b}_{h}", tag=f"qT{b}_{h}") for h in range(QH)]
                for b in range(B)
            ]
            kT = [pp.tile([P, S], BF16, name=f"kT{b}", tag=f"kT{b}") for b in range(B)]
            vB = [pp.tile([P, S], BF16, name=f"vB{b}", tag=f"vB{b}") for b in range(B)]

            # ---- prologue: load + transpose/cast ----
            with (
                tc.tile_pool(name="load", bufs=4) as loadp,
                tc.tile_pool(name="tpsum", bufs=2, space="PSUM") as tpp,
            ):
                for b in range(B):
                    for t in range(NT):
                        rows = slice(t * P, (t + 1) * P)
                        cols = slice(t * P, (t + 1) * P)
                        ktile = loadp.tile([P, P], F32, name="kload", tag="kload")
                        nc.sync.dma_start(out=ktile[:], in_=k_ext[b, rows, :])
                        kps = tpp.tile([P, P], F32, name="tp", tag="tp")
                        nc.tensor.transpose(kps[:], ktile[:], ident[:])
                        nc.vector.tensor_copy(kT[b][:, cols], kps[:])

                        vtile = loadp.tile([P, P], F32, name="vload", tag="vload")
                        nc.sync.dma_start(out=vtile[:], in_=v_ext[b, rows, :])
                        nc.vector.tensor_copy(vB[b][:, cols], vtile[:])

                        qtile = loadp.tile([P, QH * P], F32, name="qload", tag="qload")
                        nc.sync.dma_start(out=qtile[:], in_=q_ext[b, rows, :])
                        for h in range(QH):
                            qps = tpp.tile([P, P], F32, name="tp", tag="tp")
                            nc.tensor.transpose(
                                qps[:], qtile[:, h * P : (h + 1) * P], ident[:]
                            )
                            nc.vector.tensor_copy(qT[b][h][:, cols], qps[:])

            # ---- main loop ----
            with (
                tc.tile_pool(name="spsum", bufs=2, space="PSUM") as sp,
                tc.tile_pool(name="apsum", bufs=2, space="PSUM") as auxp,
                tc.tile_pool(name="tbuf", bufs=2) as tbp,
                tc.tile_pool(name="ebuf", bufs=13) as ebp,
                tc.tile_pool(name="misc", bufs=2) as mp,
            ):
                for b in range(B):
                    for h in range(QH):
                        estrips = {}
                        for kt in range(NT):
                            w = _strip_width(kt)
                            q0s = kt * P  # strip q origin
                            strip = sp.tile([P, MAXW], F32, name="strip", tag="strip")
                            for c0 in range(0, w, CHUNK):
                                c1 = min(c0 + CHUNK, w)
                                nc.tensor.matmul(
                                    strip[:, c0:c1],
                                    lhsT=kT[b][:, q0s : q0s + P],
                                    rhs=qT[b][h][:, q0s + c0 : q0s + c1],
                                    start=True,
                                    stop=True,
                                )
                            tstrip = tbp.tile([P, MAXW], F32, name="t", tag="t")
                            nc.scalar.activation(
                                tstrip[:, :w], strip[:, :w], AF.Tanh, scale=SCALE / CAP
                            )
                            estrip = ebp.tile([P, MAXW], BF16, name="e", tag="e")
                            nc.scalar.activation(
                                estrip[:, :w], tstrip[:, :w], AF.Exp, scale=CAP
                            )
                            nc.vector.tensor_mul(estrip[:, 0:P], estrip[:, 0:P], m1[:])
                            if w > W:
                                nc.vector.tensor_mul(
                                    estrip[:, W : W + P], estrip[:, W : W + P], m2[:]
                                )
                            estrips[kt] = estrip

                            if kt % 4 != 3:
                                continue
                            # q-chunk complete
                            c = kt // 4
                            q0 = c * CHUNK
                            kts = list(range(max(0, 4 * c - 8), kt + 1))
                            num = auxp.tile([P, CHUNK], F32, name="num", tag="aux")
                            den = auxp.tile([32, CHUNK], F32, name="den", tag="aux")
                            for which, dst in (("num", num), ("den", den)):
                                for i, k2 in enumerate(kts):
                                    s0 = max(q0, k2 * P)
                                    s1 = min(q0 + CHUNK, k2 * P + _strip_width(k2))
                                    col0 = s0 - k2 * P
                                    n = s1 - s0
                                    d0 = s0 - q0
                                    lhs = (
                                        vB[b][:, k2 * P : (k2 + 1) * P]
                                        if which == "num"
                                        else ones_bf[:]
                                    )
                                    nc.tensor.matmul(
                                        dst[: lhs.shape[1], d0 : d0 + n],
                                        lhsT=lhs,
                                        rhs=estrips[k2][:, col0 : col0 + n],
                                        start=(i == 0),
                                        stop=(i == len(kts) - 1),
                                    )
                            # Drain PSUM with plain copies (no recip on PE's
                            # critical path), transpose on PE, divide after.
                            num_sb = mp.tile([P, CHUNK], F32, name="num_sb", tag="num_sb")
                            nc.vector.tensor_copy(num_sb[:], num[:])
                            den_sb = mp.tile([32, CHUNK], F32, name="den_sb", tag="den_sb")
                            nc.vector.tensor_copy(den_sb[:], den[:])
                            numt = auxp.tile([P, CHUNK], F32, name="numt", tag="aux")
                            for i in range(CHUNK // P):
                                nc.tensor.transpose(
                                    numt[:, i * P : (i + 1) * P],
                                    num_sb[:, i * P : (i + 1) * P],
                                    ident[:],
                                )
                            dent = auxp.tile([P, 4 * 32], F32, name="dent", tag="aux")
                            for i in range(CHUNK // P):
                                nc.tensor.transpose(
                                    dent[:, i * 32 : (i + 1) * 32],
                                    den_sb[:, i * P : (i + 1) * P],
                                    ident[0:32, 0:32],
                                )
                            recip = mp.tile([P, 4 * 32], F32, name="recip", tag="recip")
                            nc.vector.reciprocal_approx_fast(recip[:], dent[:])
                            ostage = mp.tile([P, CHUNK], F32, name="ostage", tag="ostage")
                            for i in range(CHUNK // P):
                                nc.vector.tensor_scalar_mul(
                                    ostage[:, i * P : (i + 1) * P],
                                    numt[:, i * P : (i + 1) * P],
                                    recip[:, i * 32 : i * 32 + 1],
                                )
                                nc.sync.dma_start(
                                    out=out_ext[
                                        b,
                                        q0 + i * P : q0 + (i + 1) * P,
                                        h * P : (h + 1) * P,
                                    ],
                                    in_=ostage[:, i * P : (i + 1) * P],
                                )
    nc.compile()
    return nc


_NC_CACHE = [None]


def _get_nc():
    if _NC_CACHE[0] is None:
        _NC_CACHE[0] = build_core_graph()
    return _NC_CACHE[0]


def _shard(query, key, value):
    in_maps = []
    for c in range(N_CORES):
        in_maps.append(
            {
                "query": np.ascontiguousarray(
                    query[:, :, c * QH * P : (c + 1) * QH * P], dtype=np.float32
                ),
                "key": np.ascontiguousarray(
                    key[:, :, c * P : (c + 1) * P], dtype=np.float32
                ),
                "value": np.ascontiguousarray(
                    value[:, :, c * P : (c + 1) * P], dtype=np.float32
                ),
            }
        )
    return in_maps


def _run(query, key, value, trace=False):
    nc = _get_nc()
    in_maps = _shard(query, key, value)
    res = run_bass_kernel_spmd(nc, in_maps, core_ids=list(range(N_CORES)), trace=trace)
    out = np.empty((B, S, N_CORES * QH * P), dtype=np.float32)
    for c in range(N_CORES):
        out[:, :, c * QH * P : (c + 1) * QH * P] = res.results[c]["out"]
    return out, res


def kernel(query, key, value):
    out, _ = _run(query, key, value, trace=False)
    return out
